# revision 10
# baseline (speedup 1.0000x reference)
"""DenseGrid multi-resolution 1-D linear interpolation on 8 Trainium2 cores.

Math: out[n, l, f] = (1-fr)*storage[off_l + i0, f] + fr*storage[off_l + i0 + 1, f]
with i0 = floor(x[n]*(R_l-1)), fr = frac(x[n]*(R_l-1)).

Device algorithm (per core, data-parallel over N):
  The whole lookup+lerp is one matmul against "tent" (hat) basis values:
      out[ch=(l,f), n] = sum_{l,j} tent(m_l*x_n - j) * storage[off_l + j, f]
  where tent(v) = relu(1 - |v|) and m_l = R_l - 1.
  1. PE:  psA[(l,j)-row, n] = m_l*(xh_n + xl_n) - j  (K=3 fp16 matmul with a
          ones row; xh/xl is a lossless hi/lo split of fp32 x, so psA is
          exact to ~2^-23; the three 128-row chunks run concurrently via PE
          row tiling)
  2. DVE + ACT: T = relu(1 - |psA|), fp16, split across both engines
  3. PE:  psO[n-part, ch] = T.T @ table             (K=320 over 3 chunks)
  4. DVE+ACT: psO -> SBUF (split), DMA out (n-major rows, contiguous)
Tables are host-side layout prep of the tiny (320x4) storage tensor,
replicated to all cores (data-parallel sharding over points).
"""

import numpy as np

import concourse.bacc as bacc
import concourse.mybir as mybir
import concourse.tile as tile
from concourse.bass_utils import run_bass_kernel_spmd

# ----------------------------------------------------------------------------
# Problem constants (hardcoded per spec)
# ----------------------------------------------------------------------------
N_FULL = 1_048_576
LEVELS = 16
FEAT = 4
N_CORES = 8
NCP = N_FULL // N_CORES            # points per core = 131072
P = 128                            # SBUF partitions
IP = NCP // P                      # i-slots per partition = 1024
RESOLUTIONS = [2 * i + 1 for i in range(2, LEVELS + 2)]   # [5,7,...,35]
KROWS = sum(RESOLUTIONS)           # 320 tent rows
KPAD = 384                         # padded to 3 chunks of 128
KCH = KPAD // P                    # 3 contract chunks

CHUNK = 1024                       # points per inner chunk (fp16 moving max)
GI = CHUNK // P                    # 128-pt groups per chunk = 8
SUPER_I = 64                       # i-slots per super-chunk (output DMA batch)

# nonlinearity split: DVE takes K-chunks 0..1, ACT takes K-chunk 2 (2-pass)
# and the PSUM->SBUF copy is split DVE_COPY cols : rest on ACT
DVE_COPY = 128                     # of GI*64 = 512 copy columns

# ----------------------------------------------------------------------------
# Custom DVE op: tent(v) = relu(1 - |v|)
# ----------------------------------------------------------------------------
_TENT_NAME = "TENT0_ANT_DG"


def _register_tent_op():
    from concourse import dve_ops
    from concourse.dve_spec import Spec, Src0, One, Zero, relu, maxx, lower
    from concourse.dve_table_gen import DveOpSpec

    if any(op.name == _TENT_NAME for op in dve_ops.OPS):
        return next(op for op in dve_ops.OPS if op.name == _TENT_NAME)

    body = relu(One - maxx(Src0, Zero - Src0))
    spec = Spec(
        body=body,
        reference=lambda in0, in1, s0, s1, imm2: np.maximum(
            1.0 - np.abs(np.asarray(in0, np.float32)), 0.0
        ),
    )
    shas = {}
    for ver in ("v3", "v4"):
        s = DveOpSpec(name=_TENT_NAME, opcode=0, uops=lower(spec, ver=ver), rd1_en=False)
        shas[ver] = s.sha(ver)
    op = dve_ops.DveOp(_TENT_NAME, spec, subdim=False, uops_sha=shas)
    dve_ops.OPS.append(op)
    dve_ops._SUB_OPCODE_FOR_NAME[op.name] = (
        dve_ops._CUSTOM_DVE_ROW_BASE + len(dve_ops.OPS) - 1
    )
    dve_ops.CUSTOM_DVE_SPECS[op.name] = op.spec
    return op


# ----------------------------------------------------------------------------
# Host table prep (tiny: 320x4 -> packed SBUF layouts; pure layout/dtype work)
# ----------------------------------------------------------------------------
def make_tables(storage, resolutions):
    storage = np.asarray(storage, np.float32)
    res = np.asarray(resolutions, np.int64)
    offs = np.concatenate([[0], np.cumsum(res)[:-1]])
    row_m = np.zeros(KPAD, np.float32)
    row_j = np.full(KPAD, 2.0, np.float32)      # pad rows: tent(0*x-2) = 0
    mvals = np.zeros((KPAD, FEAT * LEVELS), np.float32)   # [krow, ch]
    r = 0
    for l in range(LEVELS):
        m = int(res[l]) - 1
        for j in range(int(res[l])):
            row_m[r] = m
            row_j[r] = j
            mvals[r, 4 * l : 4 * l + 4] = storage[offs[l] + j]
            r += 1
    assert r == KROWS

    # affine stationary rows (32k, 32k+1, 32k+2) = (m, m, -j) for K-chunk k
    mstat = np.zeros((P, P), np.float16)
    for k in range(KCH):
        mstat[32 * k, :] = row_m[k * P : (k + 1) * P]
        mstat[32 * k + 1, :] = row_m[k * P : (k + 1) * P]
        mstat[32 * k + 2, :] = -row_j[k * P : (k + 1) * P]
    mv = np.zeros((P, KCH * 64), np.float16)               # [r_local, k*64+ch]
    for k in range(KCH):
        mv[:, k * 64 : (k + 1) * 64] = mvals[k * P : (k + 1) * P].astype(np.float16)
    return mstat, mv


# ----------------------------------------------------------------------------
# Bass program (SPMD, one program for all cores)
# ----------------------------------------------------------------------------
def build_program(ncp=NCP):
    tent_op = _register_tent_op()
    ip = ncp // P                       # i-slots
    n_super = max(1, ip // SUPER_I)
    super_i = ip // n_super             # i-slots per super-chunk
    chunks_per_super = super_i // GI
    sup_pts = super_i * P               # points per super-chunk

    f32 = mybir.dt.float32
    f16 = mybir.dt.float16
    AF = mybir.ActivationFunctionType

    nc = bacc.Bacc()
    x_ext = nc.declare_dram_parameter("x", [3, ncp], f16, isOutput=False)
    mstat_ext = nc.declare_dram_parameter("mstat", [P, P], f16, isOutput=False)
    mv_ext = nc.declare_dram_parameter("mv", [P, KCH * 64], f16, isOutput=False)
    out_ext = nc.declare_dram_parameter("out", [P, ip, 64], f32, isOutput=True)

    with tile.TileContext(nc) as tc:
        with (
            tc.tile_pool(name="consts", bufs=1) as cpool,
            tc.tile_pool(name="xin", bufs=2) as xpool,
            tc.tile_pool(name="tent", bufs=3) as tpool,
            tc.tile_pool(name="absb", bufs=3) as apool,
            tc.tile_pool(name="obuf", bufs=2) as opool,
            tc.tile_pool(name="psA", bufs=1, space="PSUM") as psa_pool,
            tc.tile_pool(name="psO", bufs=2, space="PSUM") as pso_pool,
        ):
            mstat_t = cpool.tile([P, P], f16, tag="mstat")
            mv_t = cpool.tile([P, KCH * 64], f16, tag="mv")
            nc.sync.dma_start(out=mstat_t[:], in_=mstat_ext[:])
            nc.sync.dma_start(out=mv_t[:], in_=mv_ext[:])

            for s in range(n_super):
                # x rows (xh, xl, ones) replicated at partitions (32k..32k+2)
                x_t = xpool.tile([67, sup_pts], f16, tag="x", name=f"x_{s}")
                for k in range(KCH):
                    nc.sync.dma_start(
                        out=x_t[32 * k : 32 * k + 3, :],
                        in_=x_ext[:, s * sup_pts : (s + 1) * sup_pts],
                    )
                o_t = opool.tile([P, super_i * 64], f32, tag="o", name=f"o_{s}")
                for cl in range(chunks_per_super):
                    psA = [
                        psa_pool.tile([P, CHUNK], f32, tag=f"A{k}", name=f"psA{k}_{s}_{cl}")
                        for k in range(KCH)
                    ]
                    T = [
                        tpool.tile([P, CHUNK], f16, tag=f"T{k}", name=f"T{k}_{s}_{cl}")
                        for k in range(KCH)
                    ]
                    for k in range(KCH):
                        for h in range(2):  # fp32 PSUM: max 512 cols per matmul
                            xs = slice(cl * CHUNK + h * 512, cl * CHUNK + (h + 1) * 512)
                            nc.tensor.matmul(
                                psA[k][:, h * 512 : (h + 1) * 512],
                                lhsT=mstat_t[32 * k : 32 * k + 3, :],
                                rhs=x_t[32 * k : 32 * k + 3, xs],
                                start=True,
                                stop=True,
                                tile_position=(32 * k, 0),
                            )
                    # tent nonlinearity: K-chunks 0,1 on DVE (fused custom op),
                    # K-chunk 2 on ACT (Abs then Relu) to balance engine load
                    for k in range(2):
                        nc.vector._custom_dve(tent_op, out=T[k][:], in0=psA[k][:])
                    abs_t = apool.tile([P, CHUNK], f32, tag="abs", name=f"abs_{s}_{cl}")
                    nc.scalar.activation(abs_t[:], psA[2][:], AF.Abs)
                    nc.scalar.activation(T[2][:], abs_t[:], AF.Relu, bias=1.0, scale=-1.0)

                    psO = pso_pool.tile([P, GI * 64], f32, tag="O", name=f"psO_{s}_{cl}")
                    for g in range(GI):
                        for k in range(KCH):
                            nc.tensor.matmul(
                                psO[:, g * 64 : (g + 1) * 64],
                                lhsT=T[k][:, g * P : (g + 1) * P],
                                rhs=mv_t[:, k * 64 : (k + 1) * 64],
                                start=(k == 0),
                                stop=(k == KCH - 1),
                            )
                    oc = cl * GI * 64
                    nc.vector.tensor_copy(
                        o_t[:, oc : oc + DVE_COPY], psO[:, 0:DVE_COPY]
                    )
                    nc.scalar.copy(
                        o_t[:, oc + DVE_COPY : oc + GI * 64], psO[:, DVE_COPY:]
                    )
                nc.sync.dma_start(
                    out=out_ext[:, s * super_i : (s + 1) * super_i, :],
                    in_=o_t[:],
                )
    nc.finalize()
    return nc


# ----------------------------------------------------------------------------
# Host entry point
# ----------------------------------------------------------------------------
def _proc_order(x_shard):
    """Permute points into the device processing order n' = c*CHUNK + g*128 + q
    (point = q*IP + c*GI + g), then split fp32 x losslessly into an fp16
    (hi, lo) pair for the PE's fp16 datapath. Pure layout/precision prep."""
    ncp = x_shard.shape[0]
    ip = ncp // P
    xp = np.ascontiguousarray(
        x_shard.reshape(P, ip // GI, GI).transpose(1, 2, 0)
    ).reshape(-1)
    xh = xp.astype(np.float16)
    xl = (xp - xh.astype(np.float32)).astype(np.float16)
    ones = np.ones_like(xh)
    return np.stack([xh, xl, ones])


_PROGRAM_CACHE = {}


def kernel(x, storage, resolutions):
    x = np.asarray(x, np.float32).reshape(-1)
    assert x.shape[0] == N_FULL
    mstat, mv = make_tables(storage, resolutions)

    if NCP not in _PROGRAM_CACHE:
        _PROGRAM_CACHE[NCP] = build_program(NCP)
    nc = _PROGRAM_CACHE[NCP]

    in_maps = []
    for c in range(N_CORES):
        shard = x[c * NCP : (c + 1) * NCP]
        in_maps.append({"x": _proc_order(shard), "mstat": mstat, "mv": mv})
    res = run_bass_kernel_spmd(nc, in_maps, list(range(N_CORES)))
    outs = [r["out"].reshape(NCP, LEVELS, FEAT) for r in res.results]
    return np.concatenate(outs, axis=0)
